# revision 22
# baseline (speedup 1.0000x reference)
"""DenseGraphAttentionHead Trainium2 Bass kernel (8-core SPMD row-sharded).

reference math:
    Wh = nodes @ W_w.T + W_b                    [N, 256]
    Wh1 = Wh @ a1_w.T + a1_b                    [N, 1]
    Wh2 = Wh @ a2_w.T + a2_b                    [N, 1]
    scores = leaky_relu(Wh1 + Wh2.T, 0.2)       [N, N]
    attention = softmax(where(edge, scores, -inf), axis=1)
    out = attention @ Wh                        [N, 256]

Key identity: softmax over j is invariant to per-row(i) factors, so with
    p[i] = exp(0.8*Wh1[i]),  q[j] = exp(0.2*Wh2[j]),  r[j] = exp(Wh2[j])
we have  exp(lrelu(Wh1+Wh2) - 0.2*Wh1) = max(q[j], r[j]*p[i])
(branch r*p >= q  <=>  Wh1+Wh2 >= 0, exactly the lrelu branch), hence
    attention_ij ∝ edge_ij * max(q[j], r[j]*p[i]).
The dense exp/lrelu over the 8192x8192 score matrix collapses to one fused
DVE tensor_scalar (mult+max) per 128-chunk, with the edge mask applied as
min(X, {0|BIG}) *during the DMA* (SWDGE CCE), and exps only on vectors.

Per core c (rows i in [c*1024, (c+1)*1024), scores in [j(part), i(free)]):
  - Wh_aug[j, 0:256] = nodes @ W_w.T (fp16, no bias), col 256 = 1 (rowsum
    column), col 257 = nodes @ v2 = Wh2-c2 (a2 folded into params host-side).
  - X[j, i] = max(q[j], r[j]*p[i]); X = min(X, mask) via accumulate-DMA.
  - psum[i, 0:258] += X[:, i_blk].T @ Wh_aug over j chunks; col 256 = softmax
    denominator. out = psum[:, :256]/denom + W_b (softmax rows sum to 1, so
    the +W_b bias commutes with attention@).
"""
import sys
import types

import numpy as np

N_NODES = 8192
IN_DIM = 512
OUT_DIM = 256
ALPHA = 0.2
N_CORES = 8
ROWS = N_NODES // N_CORES          # 1024 rows per core
NCK = N_NODES // 128               # 64 j-chunks of 128
GRP = 4                            # j-chunks per mask-DMA batch
MASK_NEG = np.float16(-28672.0)

_CACHE = {}


def _ensure_ntff_hook():
    """antenv.axon_hooks is absent in this container; shim it so
    run_bass_kernel_spmd(trace=True) can reach the NTFF profiler."""
    if "antenv.axon_hooks" in sys.modules:
        return
    holder = [None]
    mod = types.ModuleType("antenv.axon_hooks")
    mod.set_axon_ntff_profile_hook = lambda h: holder.__setitem__(0, h)
    mod.get_axon_ntff_profile_hook = lambda: holder[0]
    sys.modules["antenv.axon_hooks"] = mod
    try:
        from trn_agent_boot.trn_boot import _ntff_profile_via_ctypes
        mod.set_axon_ntff_profile_hook(
            _ntff_profile_via_ctypes("/opt/axon/libaxon_pjrt.so"))
    except Exception:
        pass


def _build_nc():
    import concourse.bacc as bacc
    import concourse.tile as tile
    from concourse import mybir

    F16 = mybir.dt.float16
    F32 = mybir.dt.float32
    ADD = mybir.AluOpType.add
    MULT = mybir.AluOpType.mult
    MAX = mybir.AluOpType.max
    MIN = mybir.AluOpType.min
    EXP = mybir.ActivationFunctionType.Exp

    nc = bacc.Bacc("TRN2", target_bir_lowering=False, debug=False,
                   num_devices=N_CORES)

    nodesT_d = nc.dram_tensor("nodesT", [IN_DIM, N_NODES], F16,
                              kind="ExternalInput")
    ndown_d = nc.dram_tensor("nodesT_own", [IN_DIM, ROWS], F16,
                             kind="ExternalInput")
    maskm_d = nc.dram_tensor("maskm", [N_NODES, ROWS], mybir.dt.float8e4,
                             kind="ExternalInput")
    wtaug_d = nc.dram_tensor("wt_aug", [IN_DIM, 258], F16,
                             kind="ExternalInput")
    v1_d = nc.dram_tensor("v1", [IN_DIM, 1], F16, kind="ExternalInput")
    wb_d = nc.dram_tensor("wb_bc", [128, OUT_DIM], F32, kind="ExternalInput")
    c1_d = nc.dram_tensor("c1", [1, 1], F32, kind="ExternalInput")
    c2_d = nc.dram_tensor("c2qr", [128, 2], F32, kind="ExternalInput")
    out_d = nc.dram_tensor("out", [ROWS, OUT_DIM], F32, kind="ExternalOutput")

    with tile.TileContext(nc) as tc:
        with (
            tc.tile_pool(name="consts", bufs=1) as consts,
            tc.tile_pool(name="ndpool", bufs=3) as ndpool,
            tc.tile_pool(name="grpp", bufs=4) as grpp,
            tc.tile_pool(name="outp", bufs=2) as outp,
        ):
            # ---- constants ----
            wt_t = []
            v1_t = []
            ndown_t = []
            for d4 in range(4):
                v = consts.tile([128, 1], F16, name=f"v1_{d4}", tag=f"v1_{d4}")
                nc.sync.dma_start(v[:], v1_d[d4 * 128:(d4 + 1) * 128, :])
                v1_t.append(v)
                nd = consts.tile([128, ROWS], F16, name=f"ndo{d4}",
                                 tag=f"ndo{d4}")
                nc.sync.dma_start(nd[:], ndown_d[d4 * 128:(d4 + 1) * 128, :])
                ndown_t.append(nd)
            for d4 in range(4):
                w = consts.tile([128, 258], F16, name=f"wt{d4}", tag=f"wt{d4}")
                nc.scalar.dma_start(w[:], wtaug_d[d4 * 128:(d4 + 1) * 128, :])
                wt_t.append(w)
            wb_bc = consts.tile([128, OUT_DIM], F32)
            nc.scalar.dma_start(wb_bc[:], wb_d[:])
            c1 = consts.tile([1, 1], F32)
            nc.sync.dma_start(c1[:], c1_d[:])
            c2qr = consts.tile([128, 2], F32)
            nc.scalar.dma_start(c2qr[:], c2_d[:])

            wh_aug = consts.tile([128, NCK, 258], F16)
            wh2f32 = consts.tile([128, NCK], F32)
            q128 = consts.tile([128, NCK], F32)
            r128 = consts.tile([128, NCK], F32)

            HALF = 512
            NG = NCK // GRP            # 16 groups of GRP chunks per half
            with (
                tc.tile_pool(name="psA", bufs=2, space="PSUM") as psA,
                tc.tile_pool(name="psB", bufs=1, space="PSUM") as psB,
            ):
                # ---- Wh1 row for own block + p = exp(0.8*Wh1) broadcast ----
                wh1row = consts.tile([1, ROWS], F16)
                for h2 in range(2):
                    pw1 = psA.tile([1, 512], F32, name="pw1", tag="pw1")
                    for d4 in range(4):
                        nc.tensor.matmul(
                            pw1[:], v1_t[d4][:],
                            ndown_t[d4][:, h2 * 512:(h2 + 1) * 512],
                            start=(d4 == 0), stop=(d4 == 3),
                            skip_group_check=True)
                    nc.vector.tensor_scalar(
                        wh1row[:, h2 * 512:(h2 + 1) * 512], pw1[:], c1[:],
                        None, op0=ADD)
                p_row = consts.tile([1, ROWS], F16)
                nc.scalar.activation(p_row[:], wh1row[:], EXP, scale=ALPHA * 4)
                p_b = consts.tile([128, ROWS], F16)
                nc.gpsimd.partition_broadcast(p_b[:], p_row[:])

                def build_wh_block(b):
                    ndT = ndpool.tile([128, 4, 1024], F16, name="ndT",
                                      tag="ndT")
                    for d4 in range(4):
                        nc.sync.dma_start(
                            ndT[:, d4, :],
                            nodesT_d[d4 * 128:(d4 + 1) * 128,
                                     b * 1024:(b + 1) * 1024])
                    for ckl in range(8):
                        ck = b * 8 + ckl
                        pwh = psA.tile([128, 258], F32, name="pwh", tag="pwh")
                        for d4 in range(4):
                            nc.tensor.matmul(
                                pwh[:],
                                ndT[:, d4, ckl * 128:(ckl + 1) * 128],
                                wt_t[d4][:],
                                start=(d4 == 0), stop=(d4 == 3),
                                skip_group_check=True)
                        # wh2 extract first (feeds q/r -> X pipeline); DVE
                        # for the first blocks where it is idle, ACT after
                        if b < 2:
                            nc.vector.tensor_copy(wh2f32[:, ck:ck + 1],
                                                  pwh[:, 257:258])
                        else:
                            nc.scalar.copy(wh2f32[:, ck:ck + 1],
                                           pwh[:, 257:258])
                        if ckl == 7:
                            sl = slice(b * 8, (b + 1) * 8)
                            nc.scalar.activation(q128[:, sl], wh2f32[:, sl],
                                                 EXP, scale=ALPHA,
                                                 bias=c2qr[:, 0:1])
                            nc.scalar.activation(r128[:, sl], wh2f32[:, sl],
                                                 EXP, scale=1.0,
                                                 bias=c2qr[:, 1:2])
                        nc.scalar.copy(wh_aug[:, ck, :], pwh[:])
                        nc.gpsimd.memset(wh_aug[:, ck, 256:257], 1.0)

                build_wh_block(0)
                build_wh_block(1)

                # ---- main sweep over i-halves, Wh blocks interleaved in
                # PE program order during the first half ----
                for h in range(2):
                    accs = [psB.tile([128, 258], F32, name=f"acc{ib}",
                                     tag=f"acc{ib}") for ib in range(4)]
                    for g in range(NG):
                        if h == 0 and g < NCK // 8 - 2:
                            build_wh_block(g + 2)
                        mgrp = grpp.tile([128, GRP, HALF], F16, name="mgrp",
                                         tag="mgrp", bufs=3)
                        msrc = maskm_d[g * GRP * 128:(g + 1) * GRP * 128,
                                       h * HALF:(h + 1) * HALF]
                        msrc = msrc.rearrange("(c p) i -> p c i", p=128)
                        nc.gpsimd.dma_start(mgrp[:], msrc)  # fp8->fp16 cast
                        sgrp = grpp.tile([128, GRP, HALF], F16, name="sgrp",
                                         tag="sgrp", bufs=4)
                        for ckl in range(GRP):
                            ck = g * GRP + ckl
                            nc.vector.tensor_scalar(
                                sgrp[:, ckl, :],
                                p_b[:, h * HALF:(h + 1) * HALF],
                                r128[:, ck:ck + 1], q128[:, ck:ck + 1],
                                op0=MULT, op1=MAX)
                        xgrp = grpp.tile([128, GRP, HALF], F16, name="xgrp",
                                         tag="xgrp", bufs=3)
                        nc.vector.tensor_tensor(xgrp[:], sgrp[:], mgrp[:],
                                                op=MULT)
                        for ckl in range(GRP):
                            ck = g * GRP + ckl
                            for ib in range(4):
                                nc.tensor.matmul(
                                    accs[ib][:],
                                    xgrp[:, ckl, ib * 128:(ib + 1) * 128],
                                    wh_aug[:, ck, :],
                                    start=(ck == 0), stop=(ck == NCK - 1),
                                    skip_group_check=True)
                    for ib in range(4):
                        recip = outp.tile([128, 1], F32, name="recip",
                                          tag="recip")
                        nc.vector.reciprocal(recip[:], accs[ib][:, 256:257])
                        o = outp.tile([128, OUT_DIM], F32, name="o", tag="o")
                        nc.vector.scalar_tensor_tensor(
                            o[:], accs[ib][:, 0:OUT_DIM], recip[:], wb_bc[:],
                            op0=MULT, op1=ADD)
                        r0 = h * HALF + ib * 128
                        nc.sync.dma_start(out_d[r0:r0 + 128, :], o[:])
    nc.compile()
    return nc


def _get_nc():
    if "nc" not in _CACHE:
        _CACHE["nc"] = _build_nc()
    return _CACHE["nc"]


def _prep_in_maps(nodes, edge_mat, W_w, W_b, a1_w, a1_b, a2_w, a2_b):
    f16 = np.float16
    nodes = np.asarray(nodes, dtype=np.float32)
    edge_mat = np.asarray(edge_mat, dtype=bool)
    W_w = np.asarray(W_w, dtype=np.float32)
    W_b = np.asarray(W_b, dtype=np.float32)
    a1_w = np.asarray(a1_w, dtype=np.float32)
    a1_b = np.asarray(a1_b, dtype=np.float32)
    a2_w = np.asarray(a2_w, dtype=np.float32)
    a2_b = np.asarray(a2_b, dtype=np.float32)

    nodesT = np.ascontiguousarray(nodes.T).astype(f16)          # [512, 8192]
    v1 = (W_w.T @ a1_w[0]).astype(f16)[:, None]                 # [512, 1]
    v2 = (W_w.T @ a2_w[0]).astype(f16)[:, None]
    wt_aug = np.concatenate(
        [W_w.T.astype(f16), np.zeros((IN_DIM, 1), f16), v2], axis=1)
    c1v = float(W_b @ a1_w[0]) + float(a1_b[0])
    c2v = float(W_b @ a2_w[0]) + float(a2_b[0])
    c1 = np.array([[c1v]], np.float32)
    c2qr = np.broadcast_to(
        np.array([ALPHA * c2v, c2v], np.float32)[None, :], (128, 2)).copy()
    wb_bc = np.ascontiguousarray(
        np.broadcast_to(W_b[None, :], (128, OUT_DIM))).astype(np.float32)
    # multiplicative {0,1} mask, transposed, fp8 (cast to fp16 during DMA)
    import ml_dtypes
    maskT = np.where(edge_mat, 1, 0).astype(ml_dtypes.float8_e4m3fn).T

    in_maps = []
    for c in range(N_CORES):
        sl = slice(c * ROWS, (c + 1) * ROWS)
        in_maps.append({
            "nodesT": nodesT,
            "nodesT_own": np.ascontiguousarray(nodesT[:, sl]),
            "maskm": np.ascontiguousarray(maskT[:, sl]),
            "wt_aug": wt_aug,
            "v1": v1,
            "wb_bc": wb_bc,
            "c1": c1,
            "c2qr": c2qr,
        })
    return in_maps


def _run(inputs, trace=False, trace_cores=None):
    from concourse.bass_utils import run_bass_kernel_spmd
    if trace:
        _ensure_ntff_hook()
    nc = _get_nc()
    in_maps = _prep_in_maps(**inputs)
    res = run_bass_kernel_spmd(nc, in_maps, list(range(N_CORES)),
                               trace=trace, trace_cores=trace_cores)
    out = np.concatenate([res.results[c]["out"] for c in range(N_CORES)],
                         axis=0)
    return out, res


def kernel(**inputs) -> np.ndarray:
    out, _ = _run(inputs, trace=False)
    return out


# revision 23
# speedup vs baseline: 1.0084x; 1.0084x over previous
"""DenseGraphAttentionHead Trainium2 Bass kernel (8-core SPMD row-sharded).

reference math:
    Wh = nodes @ W_w.T + W_b                    [N, 256]
    Wh1 = Wh @ a1_w.T + a1_b                    [N, 1]
    Wh2 = Wh @ a2_w.T + a2_b                    [N, 1]
    scores = leaky_relu(Wh1 + Wh2.T, 0.2)       [N, N]
    attention = softmax(where(edge, scores, -inf), axis=1)
    out = attention @ Wh                        [N, 256]

Key identity: softmax over j is invariant to per-row(i) factors, so with
    p[i] = exp(0.8*Wh1[i]),  q[j] = exp(0.2*Wh2[j]),  r[j] = exp(Wh2[j])
we have  exp(lrelu(Wh1+Wh2) - 0.2*Wh1) = max(q[j], r[j]*p[i])
(branch r*p >= q  <=>  Wh1+Wh2 >= 0, exactly the lrelu branch), hence
    attention_ij ∝ edge_ij * max(q[j], r[j]*p[i]).
The dense exp/lrelu over the 8192x8192 score matrix collapses to one fused
DVE tensor_scalar (mult+max) per 128-chunk plus one tensor_tensor multiply
with the {0,1} edge mask (fp8 in HBM, upcast during the SWDGE DMA); exps
only run on vectors.

Per core c (rows i in [c*1024, (c+1)*1024), scores in [j(part), i(free)]):
  - Wh_aug[j, 0:256] = nodes @ W_w.T (fp16, no bias), col 256 = 1 (rowsum
    column), col 257 = nodes @ v2 = Wh2-c2 (a2 folded into params host-side).
  - X[j, i] = max(q[j], r[j]*p[i]) * mask01[j, i].
  - psum[i, 0:258] += X[:, i_blk].T @ Wh_aug over j chunks; col 256 = softmax
    denominator. out = psum[:, :256]/denom + W_b (softmax rows sum to 1, so
    the +W_b bias commutes with attention@).
"""
import sys
import types

import numpy as np

N_NODES = 8192
IN_DIM = 512
OUT_DIM = 256
ALPHA = 0.2
N_CORES = 8
ROWS = N_NODES // N_CORES          # 1024 rows per core
NCK = N_NODES // 128               # 64 j-chunks of 128
GRP = 4                            # j-chunks per mask-DMA batch
MASK_NEG = np.float16(-28672.0)

_CACHE = {}


def _ensure_ntff_hook():
    """antenv.axon_hooks is absent in this container; shim it so
    run_bass_kernel_spmd(trace=True) can reach the NTFF profiler."""
    if "antenv.axon_hooks" in sys.modules:
        return
    holder = [None]
    mod = types.ModuleType("antenv.axon_hooks")
    mod.set_axon_ntff_profile_hook = lambda h: holder.__setitem__(0, h)
    mod.get_axon_ntff_profile_hook = lambda: holder[0]
    sys.modules["antenv.axon_hooks"] = mod
    try:
        from trn_agent_boot.trn_boot import _ntff_profile_via_ctypes
        mod.set_axon_ntff_profile_hook(
            _ntff_profile_via_ctypes("/opt/axon/libaxon_pjrt.so"))
    except Exception:
        pass


def _build_nc():
    import concourse.bacc as bacc
    import concourse.tile as tile
    from concourse import mybir

    F16 = mybir.dt.float16
    F32 = mybir.dt.float32
    ADD = mybir.AluOpType.add
    MULT = mybir.AluOpType.mult
    MAX = mybir.AluOpType.max
    MIN = mybir.AluOpType.min
    EXP = mybir.ActivationFunctionType.Exp

    nc = bacc.Bacc("TRN2", target_bir_lowering=False, debug=False,
                   num_devices=N_CORES)

    nodesT_d = nc.dram_tensor("nodesT", [IN_DIM, N_NODES], F16,
                              kind="ExternalInput")
    ndown_d = nc.dram_tensor("nodesT_own", [IN_DIM, ROWS], F16,
                             kind="ExternalInput")
    maskm_d = nc.dram_tensor("maskm", [N_NODES, ROWS], mybir.dt.float8e4,
                             kind="ExternalInput")
    wtaug_d = nc.dram_tensor("wt_aug", [IN_DIM, 258], F16,
                             kind="ExternalInput")
    v1_d = nc.dram_tensor("v1", [IN_DIM, 1], F16, kind="ExternalInput")
    wb_d = nc.dram_tensor("wb_bc", [128, OUT_DIM], F32, kind="ExternalInput")
    c1_d = nc.dram_tensor("c1", [1, 1], F32, kind="ExternalInput")
    c2_d = nc.dram_tensor("c2qr", [128, 2], F32, kind="ExternalInput")
    out_d = nc.dram_tensor("out", [ROWS, OUT_DIM], F32, kind="ExternalOutput")

    with tile.TileContext(nc) as tc:
        with (
            tc.tile_pool(name="consts", bufs=1) as consts,
            tc.tile_pool(name="ndpool", bufs=3) as ndpool,
            tc.tile_pool(name="grpp", bufs=4) as grpp,
            tc.tile_pool(name="outp", bufs=2) as outp,
        ):
            # ---- constants ----
            wt_t = []
            v1_t = []
            ndown_t = []
            for d4 in range(4):
                v = consts.tile([128, 1], F16, name=f"v1_{d4}", tag=f"v1_{d4}")
                nc.sync.dma_start(v[:], v1_d[d4 * 128:(d4 + 1) * 128, :])
                v1_t.append(v)
                nd = consts.tile([128, ROWS], F16, name=f"ndo{d4}",
                                 tag=f"ndo{d4}")
                nc.sync.dma_start(nd[:], ndown_d[d4 * 128:(d4 + 1) * 128, :])
                ndown_t.append(nd)
            for d4 in range(4):
                w = consts.tile([128, 258], F16, name=f"wt{d4}", tag=f"wt{d4}")
                nc.scalar.dma_start(w[:], wtaug_d[d4 * 128:(d4 + 1) * 128, :])
                wt_t.append(w)
            wb_bc = consts.tile([128, OUT_DIM], F32)
            nc.scalar.dma_start(wb_bc[:], wb_d[:])
            c1 = consts.tile([1, 1], F32)
            nc.sync.dma_start(c1[:], c1_d[:])
            c2qr = consts.tile([128, 2], F32)
            nc.scalar.dma_start(c2qr[:], c2_d[:])

            wh_aug = consts.tile([128, NCK, 258], F16)
            wh2f32 = consts.tile([128, NCK], F32)
            q128 = consts.tile([128, NCK], F32)
            r128 = consts.tile([128, NCK], F32)

            HALF = 512
            NG = NCK // GRP            # 16 groups of GRP chunks per half
            with (
                tc.tile_pool(name="psA", bufs=2, space="PSUM") as psA,
                tc.tile_pool(name="psB", bufs=1, space="PSUM") as psB,
            ):
                # ---- Wh1 row for own block + p = exp(0.8*Wh1) broadcast ----
                wh1row = consts.tile([1, ROWS], F16)
                for h2 in range(2):
                    pw1 = psA.tile([1, 512], F32, name="pw1", tag="pw1")
                    for d4 in range(4):
                        nc.tensor.matmul(
                            pw1[:], v1_t[d4][:],
                            ndown_t[d4][:, h2 * 512:(h2 + 1) * 512],
                            start=(d4 == 0), stop=(d4 == 3),
                            skip_group_check=True)
                    nc.vector.tensor_scalar(
                        wh1row[:, h2 * 512:(h2 + 1) * 512], pw1[:], c1[:],
                        None, op0=ADD)
                p_row = consts.tile([1, ROWS], F16)
                nc.scalar.activation(p_row[:], wh1row[:], EXP, scale=ALPHA * 4)
                p_b = consts.tile([128, ROWS], F16)
                nc.gpsimd.partition_broadcast(p_b[:], p_row[:])

                def build_wh_block(b):
                    ndT = ndpool.tile([128, 4, 1024], F16, name="ndT",
                                      tag="ndT")
                    for d4 in range(4):
                        nc.sync.dma_start(
                            ndT[:, d4, :],
                            nodesT_d[d4 * 128:(d4 + 1) * 128,
                                     b * 1024:(b + 1) * 1024])
                    for ckl in range(8):
                        ck = b * 8 + ckl
                        pwh = psA.tile([128, 258], F32, name="pwh", tag="pwh")
                        for d4 in range(4):
                            nc.tensor.matmul(
                                pwh[:],
                                ndT[:, d4, ckl * 128:(ckl + 1) * 128],
                                wt_t[d4][:],
                                start=(d4 == 0), stop=(d4 == 3),
                                skip_group_check=True)
                        # wh2 extract first (feeds q/r -> X pipeline); DVE
                        # for the first blocks where it is idle, ACT after
                        if b < 2:
                            nc.vector.tensor_copy(wh2f32[:, ck:ck + 1],
                                                  pwh[:, 257:258])
                        else:
                            nc.scalar.copy(wh2f32[:, ck:ck + 1],
                                           pwh[:, 257:258])
                        if ckl == 7:
                            sl = slice(b * 8, (b + 1) * 8)
                            nc.scalar.activation(q128[:, sl], wh2f32[:, sl],
                                                 EXP, scale=ALPHA,
                                                 bias=c2qr[:, 0:1])
                            nc.scalar.activation(r128[:, sl], wh2f32[:, sl],
                                                 EXP, scale=1.0,
                                                 bias=c2qr[:, 1:2])
                        nc.scalar.copy(wh_aug[:, ck, :], pwh[:])
                        nc.gpsimd.memset(wh_aug[:, ck, 256:257], 1.0)

                build_wh_block(0)
                build_wh_block(1)

                # ---- main sweep over i-halves, Wh blocks interleaved in
                # PE program order during the first half ----
                for h in range(2):
                    accs = [psB.tile([128, 258], F32, name=f"acc{ib}",
                                     tag=f"acc{ib}") for ib in range(4)]
                    for g in range(NG):
                        if h == 0 and g < NCK // 8 - 2:
                            build_wh_block(g + 2)
                        mgrp = grpp.tile([128, GRP, HALF], F16, name="mgrp",
                                         tag="mgrp", bufs=4)
                        msrc = maskm_d[g * GRP * 128:(g + 1) * GRP * 128,
                                       h * HALF:(h + 1) * HALF]
                        msrc = msrc.rearrange("(c p) i -> p c i", p=128)
                        nc.gpsimd.dma_start(mgrp[:], msrc)  # fp8->fp16 cast
                        sgrp = grpp.tile([128, GRP, HALF], F16, name="sgrp",
                                         tag="sgrp", bufs=4)
                        for ckl in range(GRP):
                            ck = g * GRP + ckl
                            nc.vector.tensor_scalar(
                                sgrp[:, ckl, :],
                                p_b[:, h * HALF:(h + 1) * HALF],
                                r128[:, ck:ck + 1], q128[:, ck:ck + 1],
                                op0=MULT, op1=MAX)
                        xgrp = grpp.tile([128, GRP, HALF], F16, name="xgrp",
                                         tag="xgrp", bufs=3)
                        nc.vector.tensor_tensor(xgrp[:], sgrp[:], mgrp[:],
                                                op=MULT)
                        for ckl in range(GRP):
                            ck = g * GRP + ckl
                            for ib in range(4):
                                nc.tensor.matmul(
                                    accs[ib][:],
                                    xgrp[:, ckl, ib * 128:(ib + 1) * 128],
                                    wh_aug[:, ck, :],
                                    start=(ck == 0), stop=(ck == NCK - 1),
                                    skip_group_check=True)
                    for ib in range(4):
                        recip = outp.tile([128, 1], F32, name="recip",
                                          tag="recip")
                        nc.vector.reciprocal(recip[:], accs[ib][:, 256:257])
                        o = outp.tile([128, OUT_DIM], F32, name="o", tag="o")
                        nc.vector.scalar_tensor_tensor(
                            o[:], accs[ib][:, 0:OUT_DIM], recip[:], wb_bc[:],
                            op0=MULT, op1=ADD)
                        r0 = h * HALF + ib * 128
                        nc.sync.dma_start(out_d[r0:r0 + 128, :], o[:])
    nc.compile()
    return nc


def _get_nc():
    if "nc" not in _CACHE:
        _CACHE["nc"] = _build_nc()
    return _CACHE["nc"]


def _prep_in_maps(nodes, edge_mat, W_w, W_b, a1_w, a1_b, a2_w, a2_b):
    f16 = np.float16
    nodes = np.asarray(nodes, dtype=np.float32)
    edge_mat = np.asarray(edge_mat, dtype=bool)
    W_w = np.asarray(W_w, dtype=np.float32)
    W_b = np.asarray(W_b, dtype=np.float32)
    a1_w = np.asarray(a1_w, dtype=np.float32)
    a1_b = np.asarray(a1_b, dtype=np.float32)
    a2_w = np.asarray(a2_w, dtype=np.float32)
    a2_b = np.asarray(a2_b, dtype=np.float32)

    nodesT = np.ascontiguousarray(nodes.T).astype(f16)          # [512, 8192]
    v1 = (W_w.T @ a1_w[0]).astype(f16)[:, None]                 # [512, 1]
    v2 = (W_w.T @ a2_w[0]).astype(f16)[:, None]
    wt_aug = np.concatenate(
        [W_w.T.astype(f16), np.zeros((IN_DIM, 1), f16), v2], axis=1)
    c1v = float(W_b @ a1_w[0]) + float(a1_b[0])
    c2v = float(W_b @ a2_w[0]) + float(a2_b[0])
    c1 = np.array([[c1v]], np.float32)
    c2qr = np.broadcast_to(
        np.array([ALPHA * c2v, c2v], np.float32)[None, :], (128, 2)).copy()
    wb_bc = np.ascontiguousarray(
        np.broadcast_to(W_b[None, :], (128, OUT_DIM))).astype(np.float32)
    # multiplicative {0,1} mask, transposed, fp8 (cast to fp16 during DMA)
    import ml_dtypes
    maskT = np.where(edge_mat, 1, 0).astype(ml_dtypes.float8_e4m3fn).T

    in_maps = []
    for c in range(N_CORES):
        sl = slice(c * ROWS, (c + 1) * ROWS)
        in_maps.append({
            "nodesT": nodesT,
            "nodesT_own": np.ascontiguousarray(nodesT[:, sl]),
            "maskm": np.ascontiguousarray(maskT[:, sl]),
            "wt_aug": wt_aug,
            "v1": v1,
            "wb_bc": wb_bc,
            "c1": c1,
            "c2qr": c2qr,
        })
    return in_maps


def _run(inputs, trace=False, trace_cores=None):
    from concourse.bass_utils import run_bass_kernel_spmd
    if trace:
        _ensure_ntff_hook()
    nc = _get_nc()
    in_maps = _prep_in_maps(**inputs)
    res = run_bass_kernel_spmd(nc, in_maps, list(range(N_CORES)),
                               trace=trace, trace_cores=trace_cores)
    out = np.concatenate([res.results[c]["out"] for c in range(N_CORES)],
                         axis=0)
    return out, res


def kernel(**inputs) -> np.ndarray:
    out, _ = _run(inputs, trace=False)
    return out
